# revision 55
# baseline (speedup 1.0000x reference)
"""Trainium2 Bass kernel for NeuronLlama4VisionMLP (fused residual-add +
RMSNorm + up-proj + GELU + down-proj).

Distribution: data-parallel over the 16384 tokens -> 2048 tokens per core,
full weights replicated per core, no collectives.

Host side (cheap elementwise / repack prep):
  - h = x + residual  (this is also the module's second output)
  - per-token rsqrt(mean(h^2)+eps) scale and ln_w are folded into the
    device inputs: normed = h * s, W_up' = ln_w[:,None] * W_up
  - normed is shipped transposed ([H, T] per core, fp16) so the device is
    a pure matmul pipeline; the device returns out^T (fp16) and b_down is
    added on host.

Device side per core (T=2048 tokens, H=1408, I=5632), two 1024-token
blocks, all matmuls fp16 (1 col/cycle PE rate, FWL weight loads):
    up:   psum[i_tile, tok] = sum_k W_up'[k, i_tile].T @ normed_T[k, tok]
    gelu: act[i_tile] = Gelu(psum + b_up[i_tile])       (ACT engine)
    down: psum[m_tile, tok] = sum_i W_down[i, m_tile].T @ act[i]
    out^T[m_tile, tok] (fp16) -> HBM

Schedule details aimed at keeping the PE matmul stream gapless. The
startup ramp is bound by the per-core DMA grant (~200GB/s while all 8
cores pull their first ~6MB simultaneously), so everything else is
built around covering that window with compute:
  - ~4us of dependency-free warm-up matmuls flip the HAM clock gate to
    8/8 before the first real matmul
  - block-0 prologue runs the first 8 i-chains k-outer in 2 groups of 4
    (~38us of compute), paced by [nt chunk, group-1 weight chunk] pairs
    interleaved in exact consumption order on a single DMA ring
  - DMA layouts are partition-major with multi-KB per-partition runs
    (the DMA engines are packet-rate limited: 2KB packets move ~90GB/s,
    >=4KB packets ~420GB/s)
  - one shared PSUM pool (4 x [128,1024] f32 = all 8 banks)
  - inner loops put the 512-col chunk innermost so each stationary
    weight is used by 2 consecutive matmuls (LDWEIGHTS fully hidden)
  - remaining weights stream as i-pair tiles on the sync ring with a
    4-buffer lead; GELUs own the scalar queue (a DMA trigger queued
    behind a pending GELU would head-block a whole chain)
"""
import sys

sys.path.insert(0, "/opt/trn_rl_repo")

import numpy as np
import concourse.bass as bass
from concourse import bacc
import concourse.mybir as mybir
from concourse.tile import TileContext
from concourse.bass_utils import run_bass_kernel_spmd

# Problem shape (hardcoded per contract)
B, S, H, I = 16, 1024, 1408, 5632
EPS = 1e-6
NCORES = 8
P = 128
T_CORE = (B * S) // NCORES       # 2048 tokens per core
KH = H // P                      # 11 k-tiles of H
KI = I // P                      # 44 i-tiles of I
IC = 4                           # i-chunks in down phase
ISUB = KI // IC                  # 11 i-subtiles per chunk
TB = 1024                        # tokens per block
NB = T_CORE // TB                # 2 blocks
NCH = TB // 512                  # 512-col matmul chunks per psum tile
G0 = 4                           # i-tiles per k-outer prologue group
NPG = 2                          # prologue groups (the startup ramp is
                                 # chip-DMA-grant bound for ~30us; 2 groups
                                 # give ~38us of k-outer compute to cover it)
NG0 = G0 * NPG                   # i-tiles covered by k-major weights


def build_bass():
    dt16 = mybir.dt.float16

    nc = bacc.Bacc(None, target_bir_lowering=False)

    # nt is packed block-major AND partition-major so per-partition DMA
    # runs are multi-KB: the DMA engines are packet-rate limited (~90GB/s
    # at 2KB packets vs ~420GB/s at >=4KB), and nt is on the startup
    # critical path. nt[b, p, k, t] = normed_T[k*128+p, b*TB+t]
    nt = nc.declare_dram_parameter("nt", [NB, P, KH, TB], dt16, isOutput=False)
    # prologue weights for the first NG0 i-tiles, k-major, one tensor per
    # prologue group so group 1's critical path carries only its own bytes:
    # wg0x[p, k, g, c] = Wup'[k*128+p, (x*G0+g)*128+c]
    wg0a = nc.declare_dram_parameter("wg0a", [P, KH, G0, P], dt16, isOutput=False)
    wg0b = nc.declare_dram_parameter("wg0b", [P, KH, G0, P], dt16, isOutput=False)
    # i-pair weight tiles: pair j covers i-tiles {2j, 2j+1}
    # wupr: pairs j>=NG0/2 (block-0 after the prologue), wupf: all (block 1)
    wupr = nc.declare_dram_parameter(
        "wupr", [(KI - NG0) // 2, P, 2, KH, P], dt16, isOutput=False
    )
    wupf = nc.declare_dram_parameter(
        "wupf", [KI // 2, P, 2, KH, P], dt16, isOutput=False
    )
    wdn = nc.declare_dram_parameter("wdn", [KH, P, IC, ISUB, P], dt16, isOutput=False)
    bup = nc.declare_dram_parameter("bup", [I], mybir.dt.float32, isOutput=False)
    ot = nc.declare_dram_parameter("ot", [H, T_CORE], dt16, isOutput=True)

    bup2 = bup.rearrange("(i p) -> p i", p=P)         # [128, KI]

    with TileContext(nc) as tc:
        with (
            tc.tile_pool(name="const", bufs=1) as constp,
            tc.tile_pool(name="ntp0", bufs=1) as ntp0,
            tc.tile_pool(name="ntp1", bufs=1) as ntp1,
            tc.tile_pool(name="wg0p", bufs=1) as wg0p,
            tc.tile_pool(name="wupp", bufs=4) as wupp,
            tc.tile_pool(name="wdnp", bufs=2) as wdnp,
            tc.tile_pool(name="actp", bufs=KI + 2) as actp,
            tc.tile_pool(name="outp", bufs=2) as outp,
            tc.tile_pool(name="psp", bufs=4, space="PSUM") as psp,
        ):
            bup_sb = constp.tile([P, KI], mybir.dt.float32)
            nc.gpsimd.dma_start(out=bup_sb[:], in_=bup2)

            tok0 = slice(0, TB)
            tok1 = slice(TB, 2 * TB)

            # PE warm-up: ~5us of near-dependency-free matmuls flip the HAM
            # clock gate to 8/8 while the first real DMAs are still in
            # flight, so real matmuls start warm. The memset rides the
            # otherwise-idle vector engine so the cross-engine dependency
            # resolves right after the preamble.
            scratch = constp.tile([P, P], dt16, tag="scratch")
            nc.vector.memset(scratch[:], 0.0)
            ps_w = psp.tile([P, TB], mybir.dt.float32, tag="ps", name="ps_warm")
            for _ in range(36):
                nc.tensor.matmul(
                    ps_w[:, 0:P], scratch[:], scratch[:], start=True, stop=True
                )

            # ---- startup-critical data rides ONE ring (sync), interleaved
            # in exact consumption order: [nt chunk k, group-1 weights k]
            # pairs. The early DMA grant is the ramp bottleneck, so a single
            # full-rate FIFO pipe in consumption order beats splitting
            # across rings whose individual shares don't match demand. ----
            nta = []  # nta[k] -> (chunk_tile, local k)
            wg0at = wg0p.tile([P, KH, G0, P], dt16, tag="wg0a")
            wg0bt = wg0p.tile([P, KH, G0, P], dt16, tag="wg0b")
            for k0, k1 in ((0, 2), (2, 5), (5, 8), (8, 11)):
                t = ntp0.tile([P, k1 - k0, TB], dt16, tag=f"ntb{k0}", name=f"ntc{k0}")
                nc.sync.dma_start(out=t[:], in_=nt[0, :, k0:k1, :])
                nc.sync.dma_start(out=wg0at[:, k0:k1], in_=wg0a[:, k0:k1])
                for k in range(k0, k1):
                    nta.append((t, k - k0))
            # group-2 weights follow; needed only once group 1 retires
            nc.sync.dma_start(out=wg0bt[:], in_=wg0b[:, :])

            def gelu(i, ps):
                acti = actp.tile([P, TB], dt16, tag="act", name=f"act{i}")
                nc.scalar.activation(
                    acti[:],
                    ps[:],
                    mybir.ActivationFunctionType.Gelu,
                    bias=bup_sb[:, i : i + 1],
                    scale=1.0,
                )
                return acti

            def up_chain(i, wap, ntat):
                """One i-tile accumulation chain: k outer, 512-col chunk
                inner so each stationary weight feeds 2 matmuls."""
                ps = psp.tile([P, TB], mybir.dt.float32, tag="ps", name=f"psu{i}")
                for k in range(KH):
                    for c in range(NCH):
                        cs = slice(c * 512, (c + 1) * 512)
                        nc.tensor.matmul(
                            ps[:, cs],
                            wap(k),
                            ntat(k, cs),
                            start=(k == 0),
                            stop=(k == KH - 1),
                        )
                return ps

            def down_phase(tok, act_tiles, last_block=False):
                for m in range(KH):
                    wdnb = wdnp.tile([P, IC, ISUB, P], dt16, tag="wdn")
                    nc.gpsimd.dma_start(out=wdnb[:], in_=wdn[m])
                    ps2 = psp.tile([P, TB], mybir.dt.float32, tag="ps", name=f"psd{m}")
                    rows = slice(m * P, (m + 1) * P)
                    if last_block and m == KH - 1:
                        # the very last tile runs as two sequential
                        # half-token chains so the first half's copy+DMA
                        # drains while the second half's matmuls run; the
                        # kernel tail then waits only on a half transfer
                        osb = outp.tile([P, TB], dt16, tag="osb")
                        for c in range(NCH):
                            cs = slice(c * 512, (c + 1) * 512)
                            for i in range(KI):
                                nc.tensor.matmul(
                                    ps2[:, cs],
                                    wdnb[:, i // ISUB, i % ISUB],
                                    act_tiles[i][:, cs],
                                    start=(i == 0),
                                    stop=(i == KI - 1),
                                )
                            nc.vector.tensor_copy(out=osb[:, cs], in_=ps2[:, cs])
                            nc.sync.dma_start(out=ot[rows, tok][:, cs], in_=osb[:, cs])
                    else:
                        for i in range(KI):
                            for c in range(NCH):
                                cs = slice(c * 512, (c + 1) * 512)
                                nc.tensor.matmul(
                                    ps2[:, cs],
                                    wdnb[:, i // ISUB, i % ISUB],
                                    act_tiles[i][:, cs],
                                    start=(i == 0),
                                    stop=(i == KI - 1),
                                )
                        osb = outp.tile([P, TB], dt16, tag="osb")
                        nc.vector.tensor_copy(out=osb[:], in_=ps2[:])
                        # out triggers ride the sync queue: nothing else
                        # queues behind them there, so a pending copy can't
                        # head-block the next wdn tile the way sharing
                        # gpsimd would
                        nc.sync.dma_start(out=ot[rows, tok], in_=osb[:])

            # ================= block 0 =================
            # prologue: first NG0 i-chains in k-outer groups of G0 -> 8
            # matmuls unblock per arriving [nt[k] + wg0[k]] row, and the
            # two groups give ~38us of compute to cover the DMA-bound ramp
            act_b0 = []
            for g, wgt in ((0, wg0at), (1, wg0bt)):
                ps_g = [
                    psp.tile([P, TB], mybir.dt.float32, tag="ps", name=f"psg{i}")
                    for i in range(g * G0, (g + 1) * G0)
                ]
                for k in range(KH):
                    nt_t, lk = nta[k]
                    for il in range(G0):
                        for c in range(NCH):
                            cs = slice(c * 512, (c + 1) * 512)
                            nc.tensor.matmul(
                                ps_g[il][:, cs],
                                wgt[:, k, il, :],
                                nt_t[:, lk, cs],
                                start=(k == 0),
                                stop=(k == KH - 1),
                            )
                act_b0 += [gelu(g * G0 + il, ps_g[il]) for il in range(G0)]

            # remaining i-tiles, pair-streamed weights. Triggers ride the
            # sync queue (NOT scalar: a strict-FIFO queue behind pending
            # GELUs would delay each pair by a whole chain)
            for i in range(NG0, KI):
                j = (i - NG0) // 2
                half = (i - NG0) % 2
                if half == 0:
                    wupb = wupp.tile([P, 2, KH, P], dt16, tag="wup", name=f"wupr{j}")
                    nc.sync.dma_start(out=wupb[:], in_=wupr[j])
                ps = up_chain(
                    i,
                    lambda k: wupb[:, half, k],
                    lambda k, cs: nta[k][0][:, nta[k][1], cs],
                )
                act_b0.append(gelu(i, ps))

            # block-1 nt + first two weight pairs prefetch now: queued
            # behind the buf-gated wupr triggers, they fire late in the
            # up phase and land during down-b0, long before they're read
            ntb1 = ntp1.tile([P, KH, TB], dt16, tag="ntb1")
            nc.sync.dma_start(out=ntb1[:], in_=nt[1])
            wupf_pre = []
            for j in range(4):
                t = wupp.tile([P, 2, KH, P], dt16, tag="wup", name=f"wupf{j}")
                nc.sync.dma_start(out=t[:], in_=wupf[j])
                wupf_pre.append(t)

            down_phase(tok0, act_b0)

            # ================= block 1 =================
            act_b1 = []
            for i in range(KI):
                j, half = divmod(i, 2)
                if half == 0:
                    if j < 4:
                        wupb1 = wupf_pre[j]
                    else:
                        wupb1 = wupp.tile(
                            [P, 2, KH, P], dt16, tag="wup", name=f"wupf{j}"
                        )
                        nc.sync.dma_start(out=wupb1[:], in_=wupf[j])
                ps = up_chain(
                    i, lambda k: wupb1[:, half, k], lambda k, cs: ntb1[:, k, cs]
                )
                act_b1.append(gelu(i, ps))

            down_phase(tok1, act_b1, last_block=True)

    nc.compile()
    return nc


_CACHED = {}


def _get_nc():
    if "nc" not in _CACHED:
        _CACHED["nc"] = build_bass()
    return _CACHED["nc"]


def _prep_host(x, residual, ln_w, W_up, b_up, W_down):
    """Host-side prep: h, normed^T per core (fp16), repacked fp16 weights."""
    h = x + residual                                   # [B,S,H] f32
    hf = h.reshape(-1, H)                              # [16384, H]
    var = np.mean(np.square(hf), axis=-1)              # f32
    s = 1.0 / np.sqrt(var + EPS)                       # f32
    normed = hf * s[:, None]                           # f32 (ln_w folded into W)

    Wup_p = (W_up * ln_w[:, None]).astype(np.float32)  # [H, I]
    W4 = Wup_p.reshape(KH, P, KI, P)
    # wg0x[p, k, g, c] = Wup_p[k*128+p, (x*G0+g)*128+c]
    WG0A = np.ascontiguousarray(W4[:, :, :G0, :].transpose(1, 0, 2, 3)).astype(
        np.float16
    )
    WG0B = np.ascontiguousarray(
        W4[:, :, G0:NG0, :].transpose(1, 0, 2, 3)
    ).astype(np.float16)
    # wupf[j, p, b, k, c] = Wup_p[k*128+p, (2j+b)*128+c]
    WUPF = np.ascontiguousarray(
        Wup_p.reshape(KH, P, KI // 2, 2, P).transpose(2, 1, 3, 0, 4)
    ).astype(np.float16)                               # [KI/2,P,2,KH,P]
    WUPR = np.ascontiguousarray(WUPF[NG0 // 2 :])      # pairs past the prologue
    # wdn[m, p, ic, isub, c] = W_down[(ic*ISUB+isub)*128+p, m*128+c]
    WDN = np.ascontiguousarray(
        W_down.reshape(IC, ISUB, P, KH, P).transpose(3, 2, 0, 1, 4)
    ).astype(np.float16)                               # [KH,P,IC,ISUB,P]

    in_maps = []
    for c in range(NCORES):
        ntc = normed[c * T_CORE : (c + 1) * T_CORE].T.astype(np.float16)
        # nt[b, p, k, t] = ntc[k*128+p, b*TB+t]
        ntb = np.ascontiguousarray(
            ntc.reshape(KH, P, NB, TB).transpose(2, 1, 0, 3)
        )
        in_maps.append(
            {
                "nt": ntb,
                "wg0a": WG0A,
                "wg0b": WG0B,
                "wupr": WUPR,
                "wupf": WUPF,
                "wdn": WDN,
                "bup": b_up.astype(np.float32),
            }
        )
    return h, in_maps


_RESET_DONE = {}


def _maybe_reset_device():
    """Best-effort terminal NRT reset so a previously wedged device can't
    hang the run. No-op when the axon .so or symbol is unavailable."""
    if _RESET_DONE:
        return
    _RESET_DONE["done"] = True
    try:
        import ctypes
        import jax

        jax.devices()
        lib = ctypes.CDLL("/opt/axon/libaxon_pjrt.so")
        if hasattr(lib, "axon_reset"):
            lib.axon_reset.restype = ctypes.c_int64
            lib.axon_reset()
    except Exception:
        pass


def _run(in_maps, **kw):
    _maybe_reset_device()
    nc = _get_nc()
    return run_bass_kernel_spmd(nc, in_maps, core_ids=list(range(NCORES)), **kw)


def _assemble(results, b_down):
    outs = [r["ot"].T.astype(np.float32) for r in results]  # each [T_CORE, H]
    out = np.concatenate(outs, axis=0).reshape(B, S, H)
    out = out + b_down.astype(np.float32)
    return out


def kernel(x, residual, ln_w, W_up, b_up, W_down, b_down):
    x = np.asarray(x, dtype=np.float32)
    residual = np.asarray(residual, dtype=np.float32)
    ln_w = np.asarray(ln_w, dtype=np.float32)
    W_up = np.asarray(W_up, dtype=np.float32)
    b_up = np.asarray(b_up, dtype=np.float32)
    W_down = np.asarray(W_down, dtype=np.float32)
    b_down = np.asarray(b_down, dtype=np.float32)

    h, in_maps = _prep_host(x, residual, ln_w, W_up, b_up, W_down)
    res = _run(in_maps)
    out = _assemble(res.results, b_down)
    return out, h


def kernel_traced(x, residual, ln_w, W_up, b_up, W_down, b_down, **kw):
    """Like kernel() but with NTFF tracing; returns ((out, h), results)."""
    h, in_maps = _prep_host(
        np.asarray(x, np.float32),
        np.asarray(residual, np.float32),
        np.asarray(ln_w, np.float32),
        np.asarray(W_up, np.float32),
        np.asarray(b_up, np.float32),
        np.asarray(W_down, np.float32),
    )
    res = _run(in_maps, trace=True, **kw)
    out = _assemble(res.results, np.asarray(b_down, np.float32))
    return (out, h), res
